# revision 1
# baseline (speedup 1.0000x reference)
"""ACAR head distributed Bass kernel for 8 TRN2 NeuronCores.

Sharding: data-parallel over the N=64 ROI axis (8 ROIs per core).
conv weights replicated (shipped pre-transposed + bf16 from host);
k/v attention operands all-gathered across cores per HR2O layer.

Host-side tricks:
  - conv1 (1x1 on concat([bg, tiled_actor])) decomposes into a shared
    bg matmul [512,256] + per-ROI actor matmul [512,8]; relu(A + B[n]).
  - all weights pre-transposed to [ic, oc] layout and cast to bf16 so
    device DMA is contiguous and matmuls run at bf16 rate.
"""

import math
import os

import numpy as np
import ml_dtypes

import concourse.bass as bass
import concourse.mybir as mybir
from concourse import bacc
from concourse import tile
from concourse.bass_utils import run_bass_kernel_spmd

F32 = mybir.dt.float32
BF = mybir.dt.bfloat16
AF = mybir.ActivationFunctionType

NL = 8          # local ROIs per core
NR = 8          # ranks
C1 = 1024       # C_R
HID = 512
NCH = HID // 128   # 4 chunks of hidden channels
ICH = C1 // 128    # 8 chunks of C_R
H1, W1, S1 = 16, 16, 256
HP, WP, SP = 7, 7, 49          # post-pool spatial
PH, PW, PS = 9, 9, 81          # padded post-pool spatial
DEPTH = 2
NCLS = 60
EPS = 1e-5
GN_M = HID * SP                # elements per GN group = 512*49


def build_kernel(sim=False):
    nc = bacc.Bacc("TRN2", target_bir_lowering=False, num_devices=NR)

    # ---- DRAM parameters (per-core views; all bf16 except gamma/beta) ----
    bg_d = nc.declare_dram_parameter("bg", [C1, S1], BF, False)
    act_d = nc.declare_dram_parameter("act", [C1, NL], BF, False)
    w1bg_d = nc.declare_dram_parameter("w1bg", [C1, HID], BF, False)
    w1act_d = nc.declare_dram_parameter("w1act", [C1, HID], BF, False)
    w2_d = nc.declare_dram_parameter("w2", [3, 3, HID, HID], BF, False)
    wq_d = nc.declare_dram_parameter("wq", [DEPTH, 3, 3, HID, HID], BF, False)
    wk_d = nc.declare_dram_parameter("wk", [DEPTH, 3, 3, HID, HID], BF, False)
    wv_d = nc.declare_dram_parameter("wv", [DEPTH, 3, 3, HID, HID], BF, False)
    wo_d = nc.declare_dram_parameter("wo", [DEPTH, 3, 3, HID, HID], BF, False)
    gam_d = nc.declare_dram_parameter("gam", [DEPTH, HID], F32, False)
    bet_d = nc.declare_dram_parameter("bet", [DEPTH, HID], F32, False)
    fc1_d = nc.declare_dram_parameter("fc1", [C1, HID], BF, False)
    fc2_d = nc.declare_dram_parameter("fc2", [2 * HID, NCLS], BF, False)
    out_d = nc.declare_dram_parameter("out", [NCLS, NL], F32, True)

    rg = [list(range(NR))]

    with tile.TileContext(nc) as tc:
        with (
            tc.tile_pool(name="persist", bufs=1) as pp,
            tc.tile_pool(name="wconv", bufs=5) as wp,
            tc.tile_pool(name="work", bufs=2) as wk_pool,
            tc.tile_pool(name="kv", bufs=2) as kvp,
            tc.tile_pool(name="ps_conv", bufs=2, space="PSUM") as ps_conv,
            tc.tile_pool(name="ps_att", bufs=1, space="PSUM") as ps_att,
            tc.tile_pool(name="ps_virt", bufs=2, space="PSUM") as ps_virt,
            tc.tile_pool(name="ps_small", bufs=3, space="PSUM") as ps_small,
            tc.tile_pool(name="dram", bufs=1, space="DRAM") as dp,
        ):
            # ---------------- persistent state ----------------
            x_pad = [pp.tile([128, NL, PH, PW], F32, tag=f"xpad{c}", name=f"xpad{c}")
                     for c in range(NCH)]
            x_bf = [pp.tile([128, NL, PH, PW], BF, tag=f"xbf{c}", name=f"xbf{c}")
                    for c in range(NCH)]
            v_pad = [pp.tile([128, NL, PH, PW], BF, tag=f"vpad{c}", name=f"vpad{c}")
                     for c in range(NCH)]
            for c in range(NCH):
                nc.vector.memset(x_pad[c][:], 0.0)
                nc.vector.memset(x_bf[c][:], 0.0)
                nc.vector.memset(v_pad[c][:], 0.0)

            ones_bf = pp.tile([64, 1], BF, tag="ones_bf", name="ones_bf")
            nc.vector.memset(ones_bf[:], 1.0)
            ones_f1 = pp.tile([1, 128], F32, tag="ones_f1", name="ones_f1")
            nc.vector.memset(ones_f1[:], 1.0)
            ones_b1 = pp.tile([1, 128], BF, tag="ones_b1", name="ones_b1")
            nc.vector.memset(ones_b1[:], 1.0)
            ones_fc = pp.tile([128, 1], F32, tag="ones_fc", name="ones_fc")
            nc.vector.memset(ones_fc[:], 1.0)
            zero_c = pp.tile([128, 1], F32, tag="zero_c", name="zero_c")
            nc.vector.memset(zero_c[:], 0.0)
            eps_c = pp.tile([128, 1], F32, tag="eps_c", name="eps_c")
            nc.vector.memset(eps_c[:], EPS)
            nc.const_aps.aps[(F32, 0.0)] = zero_c
            nc.const_aps.aps[(F32, EPS)] = eps_c

            act_sb = [pp.tile([128, NL], BF, tag=f"act{i}", name=f"act{i}") for i in range(ICH)]
            for i in range(ICH):
                nc.sync.dma_start(act_sb[i][:], act_d[i * 128:(i + 1) * 128, :])

            gam_sb = [[pp.tile([128, 1], F32, tag=f"gam{d}{c}", name=f"gam{d}{c}")
                       for c in range(NCH)] for d in range(DEPTH)]
            bet_sb = [[pp.tile([128, 1], F32, tag=f"bet{d}{c}", name=f"bet{d}{c}")
                       for c in range(NCH)] for d in range(DEPTH)]
            for d in range(DEPTH):
                for c in range(NCH):
                    nc.sync.dma_start(
                        gam_sb[d][c][:],
                        gam_d[d, c * 128:(c + 1) * 128].rearrange("(p o) -> p o", o=1))
                    nc.sync.dma_start(
                        bet_sb[d][c][:],
                        bet_d[d, c * 128:(c + 1) * 128].rearrange("(p o) -> p o", o=1))

            # ---------------- conv1 (decomposed 1x1) ----------------
            with tc.tile_pool(name="c1", bufs=1) as c1p:
                # x1 = relu(A[:, hw] + B[:, n]) in bf16, laid out [c, n, h, w]
                x1 = [c1p.tile([128, NL, H1, W1], BF, tag=f"x1{c}", name=f"x1{c}")
                      for c in range(NCH)]
                with tc.tile_pool(name="c1w", bufs=1) as c1w:
                    bg_sb = [c1w.tile([128, S1], BF, tag=f"bg{i}", name=f"bg{i}") for i in range(ICH)]
                    w1bg_sb = [c1w.tile([128, HID], BF, tag=f"w1b{i}", name=f"w1b{i}") for i in range(ICH)]
                    w1act_sb = [c1w.tile([128, HID], BF, tag=f"w1a{i}", name=f"w1a{i}") for i in range(ICH)]
                    for i in range(ICH):
                        sl = slice(i * 128, (i + 1) * 128)
                        nc.sync.dma_start(bg_sb[i][:], bg_d[sl, :])
                        nc.sync.dma_start(w1bg_sb[i][:], w1bg_d[sl, :])
                        nc.sync.dma_start(w1act_sb[i][:], w1act_d[sl, :])
                    for occ in range(NCH):
                        osl = slice(occ * 128, (occ + 1) * 128)
                        b_ps = ps_small.tile([128, NL], F32, tag="sm", name="b1")
                        a_ps = ps_conv.tile([128, S1], F32, tag="cps", name="a1")
                        for i in range(ICH):
                            nc.tensor.matmul(b_ps[:], w1act_sb[i][:, osl],
                                             act_sb[i][:],
                                             start=(i == 0), stop=(i == ICH - 1))
                            nc.tensor.matmul(a_ps[:], w1bg_sb[i][:, osl],
                                             bg_sb[i][:],
                                             start=(i == 0), stop=(i == ICH - 1))
                        b_sb = wk_pool.tile([128, NL], F32, tag="b1sb", name="b1sb")
                        nc.vector.tensor_copy(b_sb[:], b_ps[:])
                        for n in range(NL):
                            nc.scalar.activation(
                                x1[occ][:, n].rearrange("p h w -> p (h w)"),
                                a_ps[:], AF.Relu, bias=b_sb[:, n:n + 1])

                # ---------------- conv2 (3x3 valid) + maxpool ----------------
                w2_sb = [wp.tile([128, 9, HID], BF, tag="wconv", name="wconv") for _ in range(NCH)]
                for i in range(NCH):
                    nc.sync.dma_start(
                        w2_sb[i][:],
                        w2_d[:, :, i * 128:(i + 1) * 128, :]
                        .rearrange("ky kx p o -> p (ky kx) o"))

                with tc.tile_pool(name="mp", bufs=1) as mpp:
                    pool_in = [mpp.tile([128, NL, H1, W1], BF, tag=f"pi{c}", name=f"pi{c}")
                               for c in range(NCH)]
                    for c in range(NCH):
                        nc.vector.memset(pool_in[c][:], -1e30)
                    for occ in range(NCH):
                        for np_ in range(NL // 2):
                            ps = ps_conv.tile([128, 2, 14, 14], F32, tag="cps", name="c2")
                            first = True
                            for icc in range(NCH):
                                for ky in range(3):
                                    for kx in range(3):
                                        nc.tensor.matmul(
                                            ps[:],
                                            w2_sb[icc][:, ky * 3 + kx,
                                                       occ * 128:(occ + 1) * 128],
                                            x1[icc][:, 2 * np_:2 * np_ + 2,
                                                    ky:ky + 14, kx:kx + 14],
                                            start=first,
                                            stop=(icc == NCH - 1 and ky == 2
                                                  and kx == 2))
                                        first = False
                            nc.scalar.activation(
                                pool_in[occ][:, 2 * np_:2 * np_ + 2, 1:15, 1:15],
                                ps[:], AF.Relu)
                    # maxpool 3x3 stride 2 (pad folded into the -1e30 border)
                    for c in range(NCH):
                        xi = x_pad[c][:, :, 1:8, 1:8]
                        nc.vector.tensor_copy(
                            xi, pool_in[c][:, :, 0:13:2, 0:13:2])
                        for t in range(1, 9):
                            dy, dx = t // 3, t % 3
                            nc.vector.tensor_max(
                                xi, xi,
                                pool_in[c][:, :, dy:dy + 13:2, dx:dx + 13:2])
                    for c in range(NCH):
                        nc.vector.tensor_copy(x_bf[c][:, :, 1:8, 1:8],
                                              x_pad[c][:, :, 1:8, 1:8])

            # ---------------- HR2O layers ----------------
            for d in range(DEPTH):
                kin = dp.tile([NCH, 128, NL, HP, WP], BF, tag=f"kin{d}", name=f"kin{d}")
                kout = dp.tile([NR, NCH, 128, NL, HP, WP], BF, tag=f"kout{d}", name=f"kout{d}", addr_space="Local" if sim else "Shared")
                vin = dp.tile([NCH, 128, NL, HP, WP], BF, tag=f"vin{d}", name=f"vin{d}")
                vout = dp.tile([NR, NCH, 128, NL, HP, WP], BF, tag=f"vout{d}", name=f"vout{d}", addr_space="Local" if sim else "Shared")

                def conv3(w_sb, src_bf, occ, ps_tag):
                    """3x3 same-conv on padded input; returns psum [128,NL,7,7]."""
                    ps = ps_conv.tile([128, NL, HP, WP], F32, tag="cps", name=ps_tag)
                    first = True
                    for icc in range(NCH):
                        for ky in range(3):
                            for kx in range(3):
                                nc.tensor.matmul(
                                    ps[:],
                                    w_sb[icc][:, ky * 3 + kx,
                                              occ * 128:(occ + 1) * 128],
                                    src_bf[icc][:, :, ky:ky + 7, kx:kx + 7],
                                    start=first,
                                    stop=(icc == NCH - 1 and ky == 2 and kx == 2))
                                first = False
                    return ps

                def load_w(wdram):
                    tiles = [wp.tile([128, 9, HID], BF, tag="wconv", name="wconv")
                             for _ in range(NCH)]
                    for i in range(NCH):
                        nc.sync.dma_start(
                            tiles[i][:],
                            wdram[d][:, :, i * 128:(i + 1) * 128, :]
                            .rearrange("ky kx p o -> p (ky kx) o"))
                    return tiles

                # conv_k -> stage to bounce -> AllGather
                wk_sb = load_w(wk_d)
                for occ in range(NCH):
                    ps = conv3(wk_sb, x_bf, occ, "ck")
                    st = wk_pool.tile([128, NL, HP, WP], BF, tag="kst", name="kst")
                    nc.vector.tensor_copy(st[:], ps[:])
                    nc.sync.dma_start(kin[:][occ], st[:])
                if sim:
                    for r in range(NR):
                        nc.sync.dma_start(kout[:][r], kin[:])
                else:
                    nc.gpsimd.collective_compute(
                        "AllGather", mybir.AluOpType.bypass,
                        ins=[kin[:].opt()], outs=[kout[:].opt()], replica_groups=rg)

                wv_sb = load_w(wv_d)
                for occ in range(NCH):
                    ps = conv3(wv_sb, x_bf, occ, "cv")
                    st = wk_pool.tile([128, NL, HP, WP], BF, tag="vst", name="vst")
                    nc.vector.tensor_copy(st[:], ps[:])
                    nc.sync.dma_start(vin[:][occ], st[:])
                if sim:
                    for r in range(NR):
                        nc.sync.dma_start(vout[:][r], vin[:])
                else:
                    nc.gpsimd.collective_compute(
                        "AllGather", mybir.AluOpType.bypass,
                        ins=[vin[:].opt()], outs=[vout[:].opt()], replica_groups=rg)

                # conv_q stays local
                wq_sb = load_w(wq_d)
                q_bf = [wk_pool.tile([128, NL, SP], BF, tag=f"q{c}", name=f"q{c}")
                        for c in range(NCH)]
                for occ in range(NCH):
                    ps = conv3(wq_sb, x_bf, occ, "cq")
                    nc.vector.tensor_copy(
                        q_bf[occ][:].rearrange("p n (h w) -> p n h w", h=HP),
                        ps[:])

                # attention logits: att[m, n] per hw, accumulated over c chunks
                att_ps = ps_att.tile([64, SP * NL], F32, tag="att", name="att")
                for cch in range(NCH):
                    k_all = kvp.tile([128, NR * NL, SP], BF, tag="kall", name="kall")
                    nc.sync.dma_start(
                        k_all[:].rearrange("p (r n) s -> p r n s", r=NR),
                        kout[:][:, cch].rearrange("r p n h w -> p r n (h w)"))
                    for hw in range(SP):
                        nc.tensor.matmul(
                            att_ps[:, hw * NL:(hw + 1) * NL],
                            k_all[:, :, hw], q_bf[cch][:, :, hw],
                            start=(cch == 0), stop=(cch == NCH - 1))

                # softmax over m (keys): exp then normalize via ones-matmul sum
                att_e = wk_pool.tile([64, SP * NL], BF, tag="atte", name="atte")
                nc.scalar.activation(att_e[:], att_ps[:], AF.Exp,
                                     scale=1.0 / math.sqrt(float(HID)))
                s_ps = ps_small.tile([1, SP * NL], F32, tag="sm", name="ssum")
                nc.tensor.matmul(s_ps[:], ones_bf[:], att_e[:],
                                 start=True, stop=True)
                r_sb = wk_pool.tile([1, SP * NL], F32, tag="rsum", name="rsum")
                nc.vector.reciprocal(r_sb[:], s_ps[:])
                r_bf = wk_pool.tile([1, SP * NL], BF, tag="rbf", name="rbf")
                nc.vector.tensor_copy(r_bf[:], r_sb[:])
                rb_ps = ps_small.tile([128, SP * NL], F32, tag="sm", name="rbps")
                nc.tensor.matmul(rb_ps[:], ones_b1[:], r_bf[:],
                                 start=True, stop=True)
                rb_sb = wk_pool.tile([128, SP * NL], F32, tag="rbsb", name="rbsb")
                nc.vector.tensor_copy(rb_sb[:], rb_ps[:])

                # virt[c, n] per hw = sum_m v[m, c] * att_e[m, n]; then *recip
                virt_n = [wk_pool.tile([128, SP * NL], F32, tag=f"vn{c}", name=f"vn{c}")
                          for c in range(NCH)]
                st1 = ps_small.tile([1, SP * NL], F32, tag="sm", name="st1")
                st2 = ps_small.tile([1, SP * NL], F32, tag="sm", name="st2")
                sq = wk_pool.tile([128, SP * NL], F32, tag="sq", name="sq")
                for cch in range(NCH):
                    v_all = kvp.tile([64, 128, SP], BF, tag="vall", name="vall")
                    for r in range(NR):
                        nc.sync.dma_start(
                            v_all[r * NL:(r + 1) * NL],
                            vout[:][r, cch].rearrange("p n h w -> n p (h w)"))
                    vp = ps_virt.tile([128, SP * NL], F32, tag="vps", name="vps")
                    for hw in range(SP):
                        nc.tensor.matmul(
                            vp[:, hw * NL:(hw + 1) * NL],
                            v_all[:, :, hw], att_e[:, hw * NL:(hw + 1) * NL],
                            start=True, stop=True)
                    nc.vector.tensor_mul(virt_n[cch][:], vp[:], rb_sb[:])
                    # GN stats (f32 matmul with ones over partitions)
                    nc.tensor.matmul(st1[:], ones_fc[:], virt_n[cch][:],
                                     start=(cch == 0), stop=(cch == NCH - 1))
                    nc.vector.tensor_mul(sq[:], virt_n[cch][:], virt_n[cch][:])
                    nc.tensor.matmul(st2[:], ones_fc[:], sq[:],
                                     start=(cch == 0), stop=(cch == NCH - 1))

                # finish GN stats: reduce over hw -> [1, NL]
                mu = wk_pool.tile([1, NL], F32, tag="mu", name="mu")
                m2 = wk_pool.tile([1, NL], F32, tag="m2", name="m2")
                nc.vector.reduce_sum(
                    mu[:], st1[:].rearrange("p (s n) -> p n s", n=NL),
                    axis=mybir.AxisListType.X)
                nc.vector.reduce_sum(
                    m2[:], st2[:].rearrange("p (s n) -> p n s", n=NL),
                    axis=mybir.AxisListType.X)
                nc.vector.tensor_scalar_mul(mu[:], mu[:], 1.0 / GN_M)
                nc.vector.tensor_scalar_mul(m2[:], m2[:], 1.0 / GN_M)
                musq = wk_pool.tile([1, NL], F32, tag="musq", name="musq")
                nc.vector.tensor_mul(musq[:], mu[:], mu[:])
                var = wk_pool.tile([1, NL], F32, tag="var", name="var")
                nc.vector.tensor_sub(var[:], m2[:], musq[:])
                sd = wk_pool.tile([1, NL], F32, tag="sd", name="sd")
                nc.scalar.activation(sd[:], var[:], AF.Sqrt, bias=EPS)
                rstd = wk_pool.tile([1, NL], F32, tag="rstd", name="rstd")
                nc.vector.reciprocal(rstd[:], sd[:])

                # broadcast mu/rstd across partitions via K=1 f32 matmul
                mu_ps = ps_small.tile([128, NL], F32, tag="sm", name="mups")
                rs_ps = ps_small.tile([128, NL], F32, tag="sm", name="rsps")
                nc.tensor.matmul(mu_ps[:], ones_f1[:], mu[:], start=True, stop=True)
                nc.tensor.matmul(rs_ps[:], ones_f1[:], rstd[:], start=True, stop=True)
                mu_b = wk_pool.tile([128, NL], F32, tag="mub", name="mub")
                rs_b = wk_pool.tile([128, NL], F32, tag="rsb", name="rsb")
                nc.vector.tensor_copy(mu_b[:], mu_ps[:])
                nc.vector.tensor_copy(rs_b[:], rs_ps[:])

                # per (cch, n): relu(gamma*rstd*x + (beta - gamma*mu*rstd))
                for cch in range(NCH):
                    cs = wk_pool.tile([128, NL], F32, tag="cs", name="cs")
                    nc.vector.tensor_scalar_mul(cs[:], rs_b[:], gam_sb[d][cch][:, 0:1])
                    tmp = wk_pool.tile([128, NL], F32, tag="cbt", name="cbt")
                    nc.vector.tensor_mul(tmp[:], mu_b[:], cs[:])
                    cb = wk_pool.tile([128, NL], F32, tag="cb", name="cb")
                    nc.scalar.activation(cb[:], tmp[:], AF.Identity,
                                         scale=-1.0, bias=bet_sb[d][cch][:, 0:1])
                    vr = virt_n[cch][:].rearrange("p (h w n) -> p n h w",
                                                  h=HP, w=WP, n=NL)
                    for n in range(NL):
                        nc.scalar.activation(
                            v_pad[cch][:, n, 1:8, 1:8], vr[:, n],
                            AF.Relu, scale=cs[:, n:n + 1], bias=cb[:, n:n + 1])

                # conv_o + residual into x_pad (and x_bf for next layer)
                wo_sb = load_w(wo_d)
                for occ in range(NCH):
                    ps = conv3(wo_sb, v_pad, occ, "co")
                    nc.vector.tensor_add(x_pad[occ][:, :, 1:8, 1:8],
                                         x_pad[occ][:, :, 1:8, 1:8], ps[:])
                    if d < DEPTH - 1:
                        nc.vector.tensor_copy(x_bf[occ][:, :, 1:8, 1:8],
                                              x_pad[occ][:, :, 1:8, 1:8])

            # ---------------- head: GAP + roi fc + final fc ----------------
            fc1_sb = [pp.tile([128, HID], BF, tag=f"f1{i}", name=f"f1{i}") for i in range(ICH)]
            for i in range(ICH):
                nc.sync.dma_start(fc1_sb[i][:], fc1_d[i * 128:(i + 1) * 128, :])
            fc2_sb = [pp.tile([128, NCLS], BF, tag=f"f2{i}", name=f"f2{i}")
                      for i in range(2 * NCH)]
            for i in range(2 * NCH):
                nc.sync.dma_start(fc2_sb[i][:], fc2_d[i * 128:(i + 1) * 128, :])

            feats = []  # 8 chunks of [128, NL] bf16: roi(4) then gap(4)
            for och in range(NCH):
                ps = ps_small.tile([128, NL], F32, tag="sm", name="roi")
                for i in range(ICH):
                    nc.tensor.matmul(ps[:],
                                     fc1_sb[i][:, och * 128:(och + 1) * 128],
                                     act_sb[i][:],
                                     start=(i == 0), stop=(i == ICH - 1))
                rb = pp.tile([128, NL], BF, tag=f"roib{och}", name=f"roib{och}")
                nc.scalar.activation(rb[:], ps[:], AF.Relu)
                feats.append(rb)
            for c in range(NCH):
                r1 = wk_pool.tile([128, NL, HP], F32, tag="gapr", name="gapr")
                nc.vector.reduce_sum(r1[:], x_pad[c][:, :, 1:8, 1:8],
                                     axis=mybir.AxisListType.X)
                r2 = wk_pool.tile([128, NL], F32, tag="gap2", name="gap2")
                nc.vector.reduce_sum(r2[:], r1[:], axis=mybir.AxisListType.X)
                gb = pp.tile([128, NL], BF, tag=f"gapb{c}", name=f"gapb{c}")
                nc.scalar.activation(gb[:], r2[:], AF.Copy, scale=1.0 / SP)
                feats.append(gb)

            out_ps = ps_small.tile([NCLS, NL], F32, tag="sm", name="ops")
            for i in range(2 * NCH):
                nc.tensor.matmul(out_ps[:], fc2_sb[i][:], feats[i][:],
                                 start=(i == 0), stop=(i == 2 * NCH - 1))
            out_sb = pp.tile([NCLS, NL], F32, tag="osb", name="osb")
            nc.vector.tensor_copy(out_sb[:], out_ps[:])
            nc.sync.dma_start(out_d[:], out_sb[:])

    nc.finalize()
    return nc


_NC_CACHE = None


def kernel(**inputs):
    global _NC_CACHE
    bg = np.ascontiguousarray(inputs["bg_feats"].reshape(C1, S1))
    actT = inputs["actor_feats"].T  # [1024, 64]
    w1 = inputs["w_conv1"][:, :, 0, 0]  # [512, 2048]
    w1bgT = np.ascontiguousarray(w1[:, :C1].T)
    w1actT = np.ascontiguousarray(w1[:, C1:].T)
    w2T = np.ascontiguousarray(inputs["w_conv2"].transpose(2, 3, 1, 0))
    wqT = np.ascontiguousarray(inputs["w_q"].transpose(0, 3, 4, 2, 1))
    wkT = np.ascontiguousarray(inputs["w_k"].transpose(0, 3, 4, 2, 1))
    wvT = np.ascontiguousarray(inputs["w_v"].transpose(0, 3, 4, 2, 1))
    woT = np.ascontiguousarray(inputs["w_o"].transpose(0, 3, 4, 2, 1))
    fc1T = np.ascontiguousarray(inputs["w_fc1"].T)
    fc2T = np.ascontiguousarray(inputs["w_fc2"].T)

    def bf(a):
        return np.ascontiguousarray(a).astype(ml_dtypes.bfloat16)

    common = dict(
        bg=bf(bg), w1bg=bf(w1bgT), w1act=bf(w1actT), w2=bf(w2T),
        wq=bf(wqT), wk=bf(wkT), wv=bf(wvT), wo=bf(woT),
        gam=np.ascontiguousarray(inputs["gamma"], dtype=np.float32),
        bet=np.ascontiguousarray(inputs["beta"], dtype=np.float32),
        fc1=bf(fc1T), fc2=bf(fc2T),
    )
    in_maps = [dict(common, act=bf(actT[:, r * NL:(r + 1) * NL]))
               for r in range(NR)]

    if _NC_CACHE is None:
        _NC_CACHE = build_kernel()
    trace = os.environ.get("KTRACE", "0") == "1"
    kw = {}
    if trace:
        kw = dict(trace=True, tmpdir=os.environ.get("KTRACE_DIR") or None)
    res = run_bass_kernel_spmd(_NC_CACHE, in_maps, core_ids=list(range(NR)), **kw)
    if trace and res.exec_time_ns is not None:
        print(f"HW exec time: {res.exec_time_ns} ns")
    outs = [np.asarray(res.results[r]["out"], dtype=np.float32).T
            for r in range(NR)]
    return np.concatenate(outs, axis=0)


if __name__ == "__main__":
    build_kernel()
    print("build OK")

